# revision 12
# baseline (speedup 1.0000x reference)
"""Trainium2 Bass kernel for nn_BackBone (LSTM backbone + fc + outer-product head).

Data-parallel over batch across 8 NeuronCores. Per core (b_loc rows):
  - history DMA'd with fp32->fp16 cast (SWDGE); transposed to feature-major
    xT[d, j, t, k, b] entirely by the DMA xbar transpose engine (fp16,
    block-transpose with 3D output), issued from the otherwise-idle Sync
    sequencer -- the PE does no transposes and no PSUM->SBUF copies exist
  - all matmuls fp16xfp16 (full issue rate; LDWEIGHTS overlaps), PSUM fp32
  - single batch group; the recurrence runs two independent 512-column chains
    (batch halves) so each half's ACT/DVE chain hides behind the other half's
    matmuls; k-major matmul order with the W_hh recurrent matmul last
  - gate bias folded into ScalarE's per-partition bias port; LSTM elementwise
    in fp16 (DVE 2x mode)
  - head split: y2 = relu(cn @ fc_w.T + fc_b) half of the einsum runs during
    the recurrence (independent of h); the h-half runs in the tail in l-pair
    chunks so stores stream immediately
  - out stores via SWDGE with fp16->fp32 cast; einsum entirely on DVE in fp16
"""
import numpy as np

import concourse.bacc as bacc
import concourse.mybir as mybir
import concourse.tile as tile
from concourse import bass_utils

F32 = mybir.dt.float32
F16 = mybir.dt.float16
AF = mybir.ActivationFunctionType

T = 20
D = 340
DP = 384          # d padded to 3 x 128 for the xbar block transpose
H = 128
G4 = 4
E = 32
L = 10
M3 = 3
DCH = [(0, 128), (128, 256), (256, 340)]
TCH = 4
N_CORES = 8


def build_program(b_loc: int):
    BG = b_loc                    # single group
    assert BG % 512 == 0
    NJ = BG // 128
    NCB = BG // 512               # column (batch-half) chains
    NTC = T // TCH

    nc = bacc.Bacc("TRN2", target_bir_lowering=False, debug=False)
    hist = nc.dram_tensor("history", (b_loc, T, D), F32, kind="ExternalInput").ap()
    cn = nc.dram_tensor("cluster_num", (b_loc, E), F32, kind="ExternalInput").ap()
    pref = nc.dram_tensor("pref", (b_loc, L, M3), F32, kind="ExternalInput").ap()
    wih = nc.dram_tensor("w_ih_t", (D, 4 * H), F16, kind="ExternalInput").ap()
    whh = nc.dram_tensor("w_hh_t", (H, 4 * H), F16, kind="ExternalInput").ap()
    bias4 = nc.dram_tensor("bias4", (H, G4), F32, kind="ExternalInput").ap()
    fcw = nc.dram_tensor("fc_w_t", (E, H), F16, kind="ExternalInput").ap()
    fcb = nc.dram_tensor("fc_b_row", (1, H), F16, kind="ExternalInput").ap()
    ones1 = nc.dram_tensor("ones_row", (1, 128), F16, kind="ExternalInput").ap()
    ident = nc.dram_tensor("ident", (128, 128), F16, kind="ExternalInput").ap()
    out = nc.dram_tensor("out", (b_loc, L * 256 * M3), F32, kind="ExternalOutput").ap()

    with tile.TileContext(nc) as tc:
        with tc.tile_pool(name="wpool", bufs=1) as wpool, \
             tc.tile_pool(name="main", bufs=1) as pool, \
             tc.tile_pool(name="psum", bufs=1, space="PSUM") as pspool:

            # ---- constants / weights (fp16 straight from host) ----
            ident_t = wpool.tile([128, 128], F16, name="ident_t")
            nc.sync.dma_start(ident_t[:], ident)
            bias_t = wpool.tile([H, G4], F32, name="bias_t")
            nc.sync.dma_start(bias_t[:], bias4)
            wih_t = []
            for k, (c0, c1) in enumerate(DCH):
                wt_ = wpool.tile([c1 - c0, 4 * H], F16, name=f"wih{k}")
                nc.sync.dma_start(wt_[:], wih[c0:c1, :])
                wih_t.append(wt_)
            whh_t = wpool.tile([H, 4 * H], F16, name="whh_t")
            nc.sync.dma_start(whh_t[:], whh)
            fcw_t = wpool.tile([E, H], F16, name="fcw_t")
            nc.sync.dma_start(fcw_t[:], fcw)
            fcb_t = wpool.tile([1, H], F16, name="fcb_t")
            nc.sync.dma_start(fcb_t[:], fcb)
            ones_t = wpool.tile([1, 128], F16, name="ones_t")
            nc.sync.dma_start(ones_t[:], ones1)

            # cn / pref loads early on the sync queue (before the transposes)
            cn_sbs, pf_tiles = {}, {}
            for j in range(NJ):
                rows = j * 128
                cn_sb = pool.tile([128, E], F32, name="cn_sb",
                                  tag="cnsb", bufs=NJ)
                nc.sync.dma_start(cn_sb[:], cn[rows:rows + 128, :])
                cn_sbs[j] = cn_sb
                pf32 = pool.tile([128, L, M3], F32, name="pf32",
                                 tag="pref32", bufs=2)
                nc.sync.dma_start(pf32[:], pref[rows:rows + 128, :, :])
                pf = pool.tile([128, L, M3], F16, name="pf",
                               tag="pref", bufs=NJ)
                nc.vector.tensor_copy(pf[:], pf32[:])
                pf_tiles[j] = pf

            # ---- persistent fp16 xT tile: [128, j, t, k, b] ----
            xt_all = pool.tile([128, NJ, T, 3, 128], F16, name="xt_all",
                               tag="xt")

            # ---- all history loads upfront (SWDGE queue) ----
            x_tiles = {}
            for tcc in range(NTC):
                for j in range(NJ):
                    x_t = pool.tile([128, TCH, DP], F16, name="x_t",
                                    tag="x", bufs=10)
                    nc.gpsimd.dma_start(
                        x_t[:, :, 0:D], hist[j * 128:(j + 1) * 128,
                                             tcc * TCH:(tcc + 1) * TCH, :])
                    x_tiles[(tcc, j)] = x_t

            # ---- all transposes on the sync queue (xbar engine) ----
            for tcc in range(NTC):
                for j in range(NJ):
                    x_t = x_tiles[(tcc, j)]
                    for ti in range(TCH):
                        t_abs = tcc * TCH + ti
                        nc.sync.dma_start(xt_all[:, j, t_abs, :, :],
                                          x_t[:, ti, :], transpose=True)

            def emit_einsum_half(j, y_half, n_off):
                rows = j * 128
                pf = pf_tiles[j]
                out3 = out[rows:rows + 128, :].rearrange(
                    "p (l nm) -> p l nm", l=L)
                for l0 in range(0, L, 2):
                    ol = pool.tile([128, 2, 128, M3], F16, name="ol",
                                   tag="outl", bufs=4)
                    y_b = y_half[:, None, :, None].broadcast_to(
                        [128, 2, 128, M3])
                    p_b = pf[:, l0:l0 + 2, None, :].broadcast_to(
                        [128, 2, 128, M3])
                    nc.vector.tensor_mul(ol[:], y_b, p_b)
                    nc.gpsimd.dma_start(
                        out3[:, l0:l0 + 2, n_off * 3:n_off * 3 + 384], ol[:])

            def emit_y2_head():
                for j in range(NJ):
                    cn_h = pool.tile([128, E], F16, name="cn_h",
                                     tag="cnh", bufs=2)
                    nc.vector.tensor_copy(cn_h[:], cn_sbs[j][:])
                    tp_c = pspool.tile([E, 128], F16, name="tp_c",
                                       tag="tp", bufs=2)
                    nc.tensor.matmul(tp_c[:], cn_h[:], ident_t[:],
                                     is_transpose=True, start=True, stop=True)
                    cnt_h = pool.tile([E, 128], F16, name="cnt_h",
                                      tag="cntr", bufs=2)
                    nc.vector.tensor_copy(cnt_h[:], tp_c[:])
                    y2p = pspool.tile([128, 128], F32, name="y2p",
                                      tag="tp", bufs=2)
                    nc.tensor.matmul(y2p[:], cnt_h[:], fcw_t[:],
                                     start=True, stop=False)
                    nc.tensor.matmul(y2p[:], ones_t[:], fcb_t[:],
                                     start=False, stop=True)
                    y2_bm = pool.tile([128, 128], F16, name="y2_bm",
                                      tag="y2bm", bufs=2)
                    nc.scalar.activation(y2_bm[:], y2p[:], AF.Relu)
                    emit_einsum_half(j, y2_bm, 128)

            def emit_h_head(h_final):
                for j in range(NJ):
                    tp_h = pspool.tile([128, 128], F16, name="tp_h",
                                       tag="tp", bufs=2)
                    nc.tensor.matmul(
                        tp_h[:],
                        h_final[:, j * 128:(j + 1) * 128],
                        ident_t[:], is_transpose=True, start=True, stop=True)
                    y1_bm = pool.tile([128, 128], F16, name="y1_bm",
                                      tag="y1bm", bufs=2)
                    nc.scalar.copy(y1_bm[:], tp_h[:])
                    emit_einsum_half(j, y1_bm, 0)

            emit_y2_head()

            # ---- fused projection + LSTM recurrence, 2 column chains ----
            h_prev = None
            c_prev = None
            for t in range(T):
                gates = [pool.tile([128, BG], F16, name=f"gate{g}",
                                   tag=f"gate{g}", bufs=2) for g in range(G4)]
                c_t = pool.tile([128, BG], F16, name="c_t", tag="c", bufs=2)
                tc_t = pool.tile([128, BG], F16, name="tc_t",
                                 tag="tanh_c", bufs=2)
                h_t = pool.tile([128, BG], F16, name="h_t", tag="h", bufs=2)
                for cb in range(NCB):
                    cs = slice(cb * 512, (cb + 1) * 512)
                    gps = [pspool.tile([128, 512], F32, name="gp",
                                       tag="gp", bufs=6) for g in range(G4)]
                    for k in range(3):
                        for g in range(G4):
                            gsl = slice(g * 128, (g + 1) * 128)
                            nc.tensor.matmul(
                                gps[g][:], wih_t[k][:, gsl],
                                xt_all[0:DCH[k][1] - DCH[k][0],
                                       cb * 4:(cb + 1) * 4, t, k, :],
                                start=(k == 0),
                                stop=(k == 2 and t == 0))
                    if t > 0:
                        for g in range(G4):
                            gsl = slice(g * 128, (g + 1) * 128)
                            nc.tensor.matmul(gps[g][:], whh_t[:, gsl],
                                             h_prev[:, cs],
                                             start=False, stop=True)
                    for g in range(G4):
                        func = AF.Tanh if g == 2 else AF.Sigmoid
                        nc.scalar.activation(gates[g][:, cs], gps[g][:], func,
                                             bias=bias_t[:, g:g + 1], scale=1.0)
                    i_t, f_t, g_t, o_t = gates
                    if t == 0:
                        nc.vector.tensor_mul(c_t[:, cs], i_t[:, cs], g_t[:, cs])
                    else:
                        t1 = pool.tile([128, 512], F16, name="t1",
                                       tag="t1", bufs=2)
                        nc.vector.tensor_mul(t1[:], f_t[:, cs], c_prev[:, cs])
                        t2 = pool.tile([128, 512], F16, name="t2",
                                       tag="t2", bufs=2)
                        nc.vector.tensor_mul(t2[:], i_t[:, cs], g_t[:, cs])
                        nc.vector.tensor_add(c_t[:, cs], t1[:], t2[:])
                    nc.scalar.activation(tc_t[:, cs], c_t[:, cs], AF.Tanh)
                    nc.vector.tensor_mul(h_t[:, cs], o_t[:, cs], tc_t[:, cs])
                h_prev, c_prev = h_t, c_t

            # ---- tail: h half of the head ----
            emit_h_head(h_prev)

    nc.compile()
    return nc


def prep_in_maps(inputs, n_cores: int, b_loc: int):
    history = np.ascontiguousarray(np.asarray(inputs["history"], np.float32))
    cluster = np.ascontiguousarray(np.asarray(inputs["cluster_num"], np.float32))
    pref = np.ascontiguousarray(np.asarray(inputs["pref"], np.float32))
    w_ih = np.asarray(inputs["W_ih"], np.float32)
    w_hh = np.asarray(inputs["W_hh"], np.float32)
    b_ih = np.asarray(inputs["b_ih"], np.float32)
    b_hh = np.asarray(inputs["b_hh"], np.float32)
    fc_w = np.asarray(inputs["fc_w"], np.float32)
    fc_b = np.asarray(inputs["fc_b"], np.float32)

    shared = {
        "w_ih_t": np.ascontiguousarray(w_ih.T.astype(np.float16)),
        "w_hh_t": np.ascontiguousarray(w_hh.T.astype(np.float16)),
        "bias4": np.ascontiguousarray((b_ih + b_hh).reshape(G4, H).T),  # [128,4]
        "fc_w_t": np.ascontiguousarray(fc_w.T.astype(np.float16)),
        "fc_b_row": np.ascontiguousarray(fc_b.reshape(1, H).astype(np.float16)),
        "ones_row": np.ones((1, 128), np.float16),
        "ident": np.eye(128, dtype=np.float16),
    }
    in_maps = []
    for c in range(n_cores):
        r0, r1 = c * b_loc, (c + 1) * b_loc
        in_maps.append({
            "history": history[r0:r1].reshape(b_loc, T, D),
            "cluster_num": cluster[r0:r1],
            "pref": pref[r0:r1].reshape(b_loc, L, M3),
            **shared,
        })
    return in_maps


def run(inputs, n_cores: int = N_CORES, trace: bool = False):
    B = np.asarray(inputs["history"]).shape[0]
    b_loc = B // n_cores
    nc = build_program(b_loc)
    in_maps = prep_in_maps(inputs, n_cores, b_loc)
    res = bass_utils.run_bass_kernel_spmd(
        nc, in_maps, core_ids=list(range(n_cores)), trace=trace)
    outs = [res.results[c]["out"].reshape(b_loc, L, 256 * M3)
            for c in range(n_cores)]
    return np.concatenate(outs, axis=0), res


def kernel(**inputs) -> np.ndarray:
    out, _ = run(inputs, N_CORES)
    return out


# revision 13
# speedup vs baseline: 1.3528x; 1.3528x over previous
"""Trainium2 Bass kernel for nn_BackBone (LSTM backbone + fc + outer-product head).

Data-parallel over batch across 8 NeuronCores. Per core (b_loc rows):
  - history DMA'd with fp32->fp16 cast (SWDGE); transposed to feature-major
    xT[d, j, t, k, b] entirely by the DMA xbar transpose engine (fp16,
    block-transpose with 3D output), issued from the otherwise-idle Sync
    sequencer -- the PE does no transposes and no PSUM->SBUF copies exist
  - all matmuls fp16xfp16 (full issue rate; LDWEIGHTS overlaps), PSUM fp32
  - single batch group; the recurrence runs two independent 512-column chains
    (batch halves) so each half's ACT/DVE chain hides behind the other half's
    matmuls; k-major matmul order with the W_hh recurrent matmul last
  - gate bias folded into ScalarE's per-partition bias port; LSTM elementwise
    in fp16 (DVE 2x mode)
  - head split: y2 = relu(cn @ fc_w.T + fc_b) half of the einsum runs during
    the recurrence (independent of h); the h-half runs in the tail in l-pair
    chunks so stores stream immediately
  - out stores via SWDGE with fp16->fp32 cast; einsum entirely on DVE in fp16
"""
import numpy as np

import concourse.bacc as bacc
import concourse.mybir as mybir
import concourse.tile as tile
from concourse import bass_utils

F32 = mybir.dt.float32
F16 = mybir.dt.float16
AF = mybir.ActivationFunctionType

T = 20
D = 340
DP = 384          # d padded to 3 x 128 for the xbar block transpose
H = 128
G4 = 4
E = 32
L = 10
M3 = 3
DCH = [(0, 128), (128, 256), (256, 340)]
TCH = 4
N_CORES = 8


def build_program(b_loc: int):
    BG = b_loc                    # single group
    assert BG % 512 == 0
    NJ = BG // 128
    NCB = BG // 512               # column (batch-half) chains
    NTC = T // TCH

    nc = bacc.Bacc("TRN2", target_bir_lowering=False, debug=False)
    hist = nc.dram_tensor("history", (b_loc, T, D), F32, kind="ExternalInput").ap()
    cn = nc.dram_tensor("cluster_num", (b_loc, E), F32, kind="ExternalInput").ap()
    pref = nc.dram_tensor("pref", (b_loc, L, M3), F32, kind="ExternalInput").ap()
    wih = nc.dram_tensor("w_ih_t", (D, 4 * H), F16, kind="ExternalInput").ap()
    whh = nc.dram_tensor("w_hh_t", (H, 4 * H), F16, kind="ExternalInput").ap()
    bias4 = nc.dram_tensor("bias4", (H, G4), F32, kind="ExternalInput").ap()
    fcw = nc.dram_tensor("fc_w_t", (E, H), F16, kind="ExternalInput").ap()
    fcb = nc.dram_tensor("fc_b_row", (1, H), F16, kind="ExternalInput").ap()
    ones1 = nc.dram_tensor("ones_row", (1, 128), F16, kind="ExternalInput").ap()
    ident = nc.dram_tensor("ident", (128, 128), F16, kind="ExternalInput").ap()
    out = nc.dram_tensor("out", (b_loc, L * 256 * M3), F32, kind="ExternalOutput").ap()

    with tile.TileContext(nc) as tc:
        with tc.tile_pool(name="wpool", bufs=1) as wpool, \
             tc.tile_pool(name="main", bufs=1) as pool, \
             tc.tile_pool(name="psum", bufs=1, space="PSUM") as pspool:

            # ---- constants / weights (fp16 straight from host) ----
            ident_t = wpool.tile([128, 128], F16, name="ident_t")
            nc.sync.dma_start(ident_t[:], ident)
            bias_t = wpool.tile([H, G4], F32, name="bias_t")
            nc.sync.dma_start(bias_t[:], bias4)
            wih_t = []
            for k, (c0, c1) in enumerate(DCH):
                wt_ = wpool.tile([c1 - c0, 4 * H], F16, name=f"wih{k}")
                nc.sync.dma_start(wt_[:], wih[c0:c1, :])
                wih_t.append(wt_)
            whh_t = wpool.tile([H, 4 * H], F16, name="whh_t")
            nc.sync.dma_start(whh_t[:], whh)
            fcw_t = wpool.tile([E, H], F16, name="fcw_t")
            nc.sync.dma_start(fcw_t[:], fcw)
            fcb_t = wpool.tile([1, H], F16, name="fcb_t")
            nc.sync.dma_start(fcb_t[:], fcb)
            ones_t = wpool.tile([1, 128], F16, name="ones_t")
            nc.sync.dma_start(ones_t[:], ones1)

            # cn / pref loads early on the sync queue (before the transposes)
            cn_sbs, pf_tiles = {}, {}
            for j in range(NJ):
                rows = j * 128
                cn_sb = pool.tile([128, E], F32, name="cn_sb",
                                  tag="cnsb", bufs=NJ)
                nc.sync.dma_start(cn_sb[:], cn[rows:rows + 128, :])
                cn_sbs[j] = cn_sb
                pf32 = pool.tile([128, L, M3], F32, name="pf32",
                                 tag="pref32", bufs=2)
                nc.sync.dma_start(pf32[:], pref[rows:rows + 128, :, :])
                pf = pool.tile([128, L, M3], F16, name="pf",
                               tag="pref", bufs=NJ)
                nc.vector.tensor_copy(pf[:], pf32[:])
                pf_tiles[j] = pf

            # ---- persistent fp16 xT tile: [128, j, t, k, b] ----
            xt_all = pool.tile([128, NJ, T, 3, 128], F16, name="xt_all",
                               tag="xt")

            # ---- all history loads upfront (SWDGE queue) ----
            x_tiles = {}
            for tcc in range(NTC):
                for j in range(NJ):
                    x_t = pool.tile([128, TCH, DP], F16, name="x_t",
                                    tag="x", bufs=10)
                    nc.gpsimd.dma_start(
                        x_t[:, :, 0:D], hist[j * 128:(j + 1) * 128,
                                             tcc * TCH:(tcc + 1) * TCH, :])
                    x_tiles[(tcc, j)] = x_t

            # ---- all transposes on the sync queue (xbar engine) ----
            # one call per (tcc, j): [128, TCH*DP] -> [128, TCH*3, 128]
            # block-transpose; blocks land exactly as the (t, k) dims of xt_all
            for tcc in range(NTC):
                for j in range(NJ):
                    x_t = x_tiles[(tcc, j)]
                    dst = xt_all[:, j, tcc * TCH:(tcc + 1) * TCH, :, :] \
                        .rearrange("p t k b -> p (t k) b")
                    nc.sync.dma_start(dst, x_t[:].rearrange("p t d -> p (t d)"),
                                      transpose=True)

            def emit_einsum_half(j, y_half, n_off):
                rows = j * 128
                pf = pf_tiles[j]
                out3 = out[rows:rows + 128, :].rearrange(
                    "p (l nm) -> p l nm", l=L)
                for l0 in range(0, L, 2):
                    ol = pool.tile([128, 2, 128, M3], F16, name="ol",
                                   tag="outl", bufs=4)
                    y_b = y_half[:, None, :, None].broadcast_to(
                        [128, 2, 128, M3])
                    p_b = pf[:, l0:l0 + 2, None, :].broadcast_to(
                        [128, 2, 128, M3])
                    nc.vector.tensor_mul(ol[:], y_b, p_b)
                    nc.gpsimd.dma_start(
                        out3[:, l0:l0 + 2, n_off * 3:n_off * 3 + 384], ol[:])

            def emit_y2_head():
                for j in range(NJ):
                    cn_h = pool.tile([128, E], F16, name="cn_h",
                                     tag="cnh", bufs=2)
                    nc.vector.tensor_copy(cn_h[:], cn_sbs[j][:])
                    tp_c = pspool.tile([E, 128], F16, name="tp_c",
                                       tag="tp", bufs=2)
                    nc.tensor.matmul(tp_c[:], cn_h[:], ident_t[:],
                                     is_transpose=True, start=True, stop=True)
                    cnt_h = pool.tile([E, 128], F16, name="cnt_h",
                                      tag="cntr", bufs=2)
                    nc.vector.tensor_copy(cnt_h[:], tp_c[:])
                    y2p = pspool.tile([128, 128], F32, name="y2p",
                                      tag="tp", bufs=2)
                    nc.tensor.matmul(y2p[:], cnt_h[:], fcw_t[:],
                                     start=True, stop=False)
                    nc.tensor.matmul(y2p[:], ones_t[:], fcb_t[:],
                                     start=False, stop=True)
                    y2_bm = pool.tile([128, 128], F16, name="y2_bm",
                                      tag="y2bm", bufs=2)
                    nc.scalar.activation(y2_bm[:], y2p[:], AF.Relu)
                    emit_einsum_half(j, y2_bm, 128)

            def emit_h_head(h_final):
                for j in range(NJ):
                    tp_h = pspool.tile([128, 128], F16, name="tp_h",
                                       tag="tp", bufs=2)
                    nc.tensor.matmul(
                        tp_h[:],
                        h_final[:, j * 128:(j + 1) * 128],
                        ident_t[:], is_transpose=True, start=True, stop=True)
                    y1_bm = pool.tile([128, 128], F16, name="y1_bm",
                                      tag="y1bm", bufs=2)
                    nc.scalar.copy(y1_bm[:], tp_h[:])
                    emit_einsum_half(j, y1_bm, 0)

            emit_y2_head()

            # ---- fused projection + LSTM recurrence, 2 column chains ----
            h_prev = None
            c_prev = None
            for t in range(T):
                gates = [pool.tile([128, BG], F16, name=f"gate{g}",
                                   tag=f"gate{g}", bufs=2) for g in range(G4)]
                c_t = pool.tile([128, BG], F16, name="c_t", tag="c", bufs=2)
                tc_t = pool.tile([128, BG], F16, name="tc_t",
                                 tag="tanh_c", bufs=2)
                h_t = pool.tile([128, BG], F16, name="h_t", tag="h", bufs=2)
                for cb in range(NCB):
                    cs = slice(cb * 512, (cb + 1) * 512)
                    gps = [pspool.tile([128, 512], F32, name="gp",
                                       tag="gp", bufs=6) for g in range(G4)]
                    for k in range(3):
                        for g in range(G4):
                            gsl = slice(g * 128, (g + 1) * 128)
                            nc.tensor.matmul(
                                gps[g][:], wih_t[k][:, gsl],
                                xt_all[0:DCH[k][1] - DCH[k][0],
                                       cb * 4:(cb + 1) * 4, t, k, :],
                                start=(k == 0),
                                stop=(k == 2 and t == 0))
                    if t > 0:
                        for g in range(G4):
                            gsl = slice(g * 128, (g + 1) * 128)
                            nc.tensor.matmul(gps[g][:], whh_t[:, gsl],
                                             h_prev[:, cs],
                                             start=False, stop=True)
                    for g in range(G4):
                        func = AF.Tanh if g == 2 else AF.Sigmoid
                        nc.scalar.activation(gates[g][:, cs], gps[g][:], func,
                                             bias=bias_t[:, g:g + 1], scale=1.0)
                    i_t, f_t, g_t, o_t = gates
                    if t == 0:
                        nc.vector.tensor_mul(c_t[:, cs], i_t[:, cs], g_t[:, cs])
                    else:
                        t1 = pool.tile([128, 512], F16, name="t1",
                                       tag="t1", bufs=2)
                        nc.vector.tensor_mul(t1[:], f_t[:, cs], c_prev[:, cs])
                        t2 = pool.tile([128, 512], F16, name="t2",
                                       tag="t2", bufs=2)
                        nc.vector.tensor_mul(t2[:], i_t[:, cs], g_t[:, cs])
                        nc.vector.tensor_add(c_t[:, cs], t1[:], t2[:])
                    nc.scalar.activation(tc_t[:, cs], c_t[:, cs], AF.Tanh)
                    nc.vector.tensor_mul(h_t[:, cs], o_t[:, cs], tc_t[:, cs])
                h_prev, c_prev = h_t, c_t

            # ---- tail: h half of the head ----
            emit_h_head(h_prev)

    nc.compile()
    return nc


def prep_in_maps(inputs, n_cores: int, b_loc: int):
    history = np.ascontiguousarray(np.asarray(inputs["history"], np.float32))
    cluster = np.ascontiguousarray(np.asarray(inputs["cluster_num"], np.float32))
    pref = np.ascontiguousarray(np.asarray(inputs["pref"], np.float32))
    w_ih = np.asarray(inputs["W_ih"], np.float32)
    w_hh = np.asarray(inputs["W_hh"], np.float32)
    b_ih = np.asarray(inputs["b_ih"], np.float32)
    b_hh = np.asarray(inputs["b_hh"], np.float32)
    fc_w = np.asarray(inputs["fc_w"], np.float32)
    fc_b = np.asarray(inputs["fc_b"], np.float32)

    shared = {
        "w_ih_t": np.ascontiguousarray(w_ih.T.astype(np.float16)),
        "w_hh_t": np.ascontiguousarray(w_hh.T.astype(np.float16)),
        "bias4": np.ascontiguousarray((b_ih + b_hh).reshape(G4, H).T),  # [128,4]
        "fc_w_t": np.ascontiguousarray(fc_w.T.astype(np.float16)),
        "fc_b_row": np.ascontiguousarray(fc_b.reshape(1, H).astype(np.float16)),
        "ones_row": np.ones((1, 128), np.float16),
        "ident": np.eye(128, dtype=np.float16),
    }
    in_maps = []
    for c in range(n_cores):
        r0, r1 = c * b_loc, (c + 1) * b_loc
        in_maps.append({
            "history": history[r0:r1].reshape(b_loc, T, D),
            "cluster_num": cluster[r0:r1],
            "pref": pref[r0:r1].reshape(b_loc, L, M3),
            **shared,
        })
    return in_maps


def run(inputs, n_cores: int = N_CORES, trace: bool = False):
    B = np.asarray(inputs["history"]).shape[0]
    b_loc = B // n_cores
    nc = build_program(b_loc)
    in_maps = prep_in_maps(inputs, n_cores, b_loc)
    res = bass_utils.run_bass_kernel_spmd(
        nc, in_maps, core_ids=list(range(n_cores)), trace=trace)
    outs = [res.results[c]["out"].reshape(b_loc, L, 256 * M3)
            for c in range(n_cores)]
    return np.concatenate(outs, axis=0), res


def kernel(**inputs) -> np.ndarray:
    out, _ = run(inputs, N_CORES)
    return out


# revision 14
# speedup vs baseline: 2.4492x; 1.8105x over previous
"""Trainium2 Bass kernel for nn_BackBone (LSTM backbone + fc + outer-product head).

Data-parallel over batch across 8 NeuronCores. Per core (b_loc rows):
  - history DMA'd with fp32->fp16 cast (SWDGE), PE-transposed (fp16, 1 cyc/row)
    to feature-major xT[d, j, t, b]; all x loads issued upfront so nothing
    ever blocks them on the GpSimd queue
  - all matmuls fp16xfp16 (full issue rate; LDWEIGHTS overlaps), PSUM fp32
  - single batch group; the recurrence runs two independent 512-column chains
    (batch halves); transposes for later t-chunks are emitted between the two
    half-blocks of each step (2 j-tiles per step) so the PE stream stays dense
  - per-step matmuls k-major with the W_hh recurrent matmul last
  - gate bias folded into ScalarE's per-partition bias port
  - head split: y2 = relu(cn @ fc_w.T + fc_b) half of the einsum runs during
    the recurrence (independent of h); the h-half runs in the tail in l-pair
    chunks so stores stream immediately
  - out stores via SWDGE with fp16->fp32 cast; einsum entirely on DVE in fp16
"""
import numpy as np

import concourse.bacc as bacc
import concourse.mybir as mybir
import concourse.tile as tile
from concourse import bass_utils

F32 = mybir.dt.float32
F16 = mybir.dt.float16
AF = mybir.ActivationFunctionType

T = 20
D = 340
H = 128
G4 = 4
E = 32
L = 10
M3 = 3
DCH = [(0, 128), (128, 256), (256, 340)]
TCH = 4           # history t-chunk (1 PSUM bank of transposes)
N_CORES = 8


def build_program(b_loc: int):
    BG = b_loc                    # single group
    assert BG % 512 == 0
    NJ = BG // 128
    NCB = BG // 512               # column (batch-half) chains
    NTC = T // TCH

    nc = bacc.Bacc("TRN2", target_bir_lowering=False, debug=False)
    hist = nc.dram_tensor("history", (b_loc, T, D), F32, kind="ExternalInput").ap()
    cn = nc.dram_tensor("cluster_num", (b_loc, E), F32, kind="ExternalInput").ap()
    pref = nc.dram_tensor("pref", (b_loc, L, M3), F32, kind="ExternalInput").ap()
    wih = nc.dram_tensor("w_ih_t", (D, 4 * H), F16, kind="ExternalInput").ap()
    whh = nc.dram_tensor("w_hh_t", (H, 4 * H), F16, kind="ExternalInput").ap()
    bias4 = nc.dram_tensor("bias4", (H, G4), F32, kind="ExternalInput").ap()
    fcw = nc.dram_tensor("fc_w_t", (E, H), F16, kind="ExternalInput").ap()
    fcb = nc.dram_tensor("fc_b_row", (1, H), F16, kind="ExternalInput").ap()
    ones1 = nc.dram_tensor("ones_row", (1, 128), F16, kind="ExternalInput").ap()
    ident = nc.dram_tensor("ident", (128, 128), F16, kind="ExternalInput").ap()
    out = nc.dram_tensor("out", (b_loc, L * 256 * M3), F32, kind="ExternalOutput").ap()

    with tile.TileContext(nc) as tc:
        with tc.tile_pool(name="wpool", bufs=1) as wpool, \
             tc.tile_pool(name="main", bufs=1) as pool, \
             tc.tile_pool(name="psum", bufs=1, space="PSUM") as pspool:

            # ---- constants / weights (fp16 straight from host) ----
            ident_t = wpool.tile([128, 128], F16, name="ident_t")
            nc.sync.dma_start(ident_t[:], ident)
            bias_t = wpool.tile([H, G4], F32, name="bias_t")
            nc.sync.dma_start(bias_t[:], bias4)
            wih_t = []
            for k, (c0, c1) in enumerate(DCH):
                wt_ = wpool.tile([c1 - c0, 4 * H], F16, name=f"wih{k}")
                nc.sync.dma_start(wt_[:], wih[c0:c1, :])
                wih_t.append(wt_)
            whh_t = wpool.tile([H, 4 * H], F16, name="whh_t")
            nc.sync.dma_start(whh_t[:], whh)
            fcw_t = wpool.tile([E, H], F16, name="fcw_t")
            nc.sync.dma_start(fcw_t[:], fcw)
            fcb_t = wpool.tile([1, H], F16, name="fcb_t")
            nc.sync.dma_start(fcb_t[:], fcb)
            ones_t = wpool.tile([1, 128], F16, name="ones_t")
            nc.sync.dma_start(ones_t[:], ones1)

            # ---- persistent fp16 xT tiles ----
            xt_tiles = [
                pool.tile([c1 - c0, NJ, T, 128], F16, name=f"xt{k}", tag=f"xt{k}")
                for k, (c0, c1) in enumerate(DCH)
            ]

            # ---- all history loads upfront (SWDGE queue, nothing blocks) ----
            x_tiles = {}
            for tcc in range(NTC):
                for j in range(NJ):
                    x_t = pool.tile([128, TCH, D], F16, name="x_t",
                                    tag="x", bufs=10)
                    nc.gpsimd.dma_start(
                        x_t[:], hist[j * 128:(j + 1) * 128,
                                     tcc * TCH:(tcc + 1) * TCH, :])
                    x_tiles[(tcc, j)] = x_t

            def emit_transposes(tcc, js):
                for j in js:
                    x_t = x_tiles[(tcc, j)]
                    for k, (c0, c1) in enumerate(DCH):
                        dk = c1 - c0
                        tp = pspool.tile([128, TCH, 128], F16, name="tp",
                                         tag="tp", bufs=2)
                        for ti in range(TCH):
                            nc.tensor.matmul(
                                tp[0:dk, ti, :],
                                x_t[:, ti, c0:c1],
                                ident_t[:],
                                is_transpose=True,
                                start=(ti == 0), stop=(ti == TCH - 1),
                            )
                        dst = xt_tiles[k][0:dk, j,
                                          tcc * TCH:(tcc + 1) * TCH, :]
                        nc.vector.tensor_copy(dst, tp[0:dk, :, :])

            pf_tiles = {}

            def emit_einsum_half(j, y_half, n_off):
                """Einsum for a 128-wide n-half of y, in l-pair chunks with a
                store per chunk (fp16 compute, SWDGE fp16->fp32 cast store)."""
                rows = j * 128
                if j not in pf_tiles:
                    pf32 = pool.tile([128, L, M3], F32, name="pf32",
                                     tag="pref32", bufs=2)
                    nc.sync.dma_start(pf32[:], pref[rows:rows + 128, :, :])
                    pf = pool.tile([128, L, M3], F16, name="pf",
                                   tag="pref", bufs=NJ)
                    nc.vector.tensor_copy(pf[:], pf32[:])
                    pf_tiles[j] = pf
                pf = pf_tiles[j]
                out3 = out[rows:rows + 128, :].rearrange(
                    "p (l nm) -> p l nm", l=L)
                for l0 in range(0, L, 2):
                    ol = pool.tile([128, 2, 128, M3], F16, name="ol",
                                   tag="outl", bufs=4)
                    y_b = y_half[:, None, :, None].broadcast_to(
                        [128, 2, 128, M3])
                    p_b = pf[:, l0:l0 + 2, None, :].broadcast_to(
                        [128, 2, 128, M3])
                    nc.vector.tensor_mul(ol[:], y_b, p_b)
                    nc.gpsimd.dma_start(
                        out3[:, l0:l0 + 2, n_off * 3:n_off * 3 + 384], ol[:])

            def emit_y2_head():
                for j in range(NJ):
                    rows = j * 128
                    cn_sb = pool.tile([128, E], F32, name="cn_sb",
                                      tag="cnsb", bufs=2)
                    nc.sync.dma_start(cn_sb[:], cn[rows:rows + 128, :])
                    cn_h = pool.tile([128, E], F16, name="cn_h",
                                     tag="cnh", bufs=2)
                    nc.vector.tensor_copy(cn_h[:], cn_sb[:])
                    tp_c = pspool.tile([E, 128], F16, name="tp_c",
                                       tag="tp", bufs=2)
                    nc.tensor.matmul(tp_c[:], cn_h[:], ident_t[:],
                                     is_transpose=True, start=True, stop=True)
                    cnt_h = pool.tile([E, 128], F16, name="cnt_h",
                                      tag="cntr", bufs=2)
                    nc.vector.tensor_copy(cnt_h[:], tp_c[:])
                    y2p = pspool.tile([128, 128], F32, name="y2p",
                                      tag="tp", bufs=2)
                    nc.tensor.matmul(y2p[:], cnt_h[:], fcw_t[:],
                                     start=True, stop=False)
                    nc.tensor.matmul(y2p[:], ones_t[:], fcb_t[:],
                                     start=False, stop=True)
                    y2_bm = pool.tile([128, 128], F16, name="y2_bm",
                                      tag="y2bm", bufs=2)
                    nc.scalar.activation(y2_bm[:], y2p[:], AF.Relu)
                    emit_einsum_half(j, y2_bm, 128)

            def emit_h_head(h_final):
                for j in range(NJ):
                    tp_h = pspool.tile([128, 128], F16, name="tp_h",
                                       tag="tp", bufs=2)
                    nc.tensor.matmul(
                        tp_h[:],
                        h_final[:, j * 128:(j + 1) * 128],
                        ident_t[:], is_transpose=True, start=True, stop=True)
                    y1_bm = pool.tile([128, 128], F16, name="y1_bm",
                                      tag="y1bm", bufs=2)
                    nc.scalar.copy(y1_bm[:], tp_h[:])
                    emit_einsum_half(j, y1_bm, 0)

            # ---- prologue: first two t-chunks + y2 head ----
            emit_transposes(0, range(NJ))
            emit_transposes(1, range(NJ))
            emit_y2_head()

            # transposes for tc 2..NTC-1 spread across steps: 2 j-tiles/step
            tp_sched = {}
            step = 3
            for tcc in range(2, NTC):
                for j0 in range(0, NJ, 2):
                    tp_sched.setdefault(step, []).append((tcc, [j0, j0 + 1]))
                    step += 1
                step = min(step, tcc * TCH + TCH + 2)

            # ---- fused projection + LSTM recurrence, 2 column chains ----
            h_prev = None
            c_prev = None
            for t in range(T):
                gates = [pool.tile([128, BG], F16, name=f"gate{g}",
                                   tag=f"gate{g}", bufs=2) for g in range(G4)]
                c_t = pool.tile([128, BG], F16, name="c_t", tag="c", bufs=2)
                tc_t = pool.tile([128, BG], F16, name="tc_t",
                                 tag="tanh_c", bufs=2)
                h_t = pool.tile([128, BG], F16, name="h_t", tag="h", bufs=2)
                for cb in range(NCB):
                    cs = slice(cb * 512, (cb + 1) * 512)
                    gps = [pspool.tile([128, 512], F32, name="gp",
                                       tag="gp", bufs=6) for g in range(G4)]
                    for k in range(3):
                        for g in range(G4):
                            gsl = slice(g * 128, (g + 1) * 128)
                            nc.tensor.matmul(
                                gps[g][:], wih_t[k][:, gsl],
                                xt_tiles[k][:, cb * 4:(cb + 1) * 4, t, :],
                                start=(k == 0),
                                stop=(k == 2 and t == 0))
                    if t > 0:
                        for g in range(G4):
                            gsl = slice(g * 128, (g + 1) * 128)
                            nc.tensor.matmul(gps[g][:], whh_t[:, gsl],
                                             h_prev[:, cs],
                                             start=False, stop=True)
                    for g in range(G4):
                        func = AF.Tanh if g == 2 else AF.Sigmoid
                        nc.scalar.activation(gates[g][:, cs], gps[g][:], func,
                                             bias=bias_t[:, g:g + 1], scale=1.0)
                    i_t, f_t, g_t, o_t = gates
                    if t == 0:
                        nc.vector.tensor_mul(c_t[:, cs], i_t[:, cs], g_t[:, cs])
                    else:
                        t1 = pool.tile([128, 512], F16, name="t1",
                                       tag="t1", bufs=2)
                        nc.vector.tensor_mul(t1[:], f_t[:, cs], c_prev[:, cs])
                        t2 = pool.tile([128, 512], F16, name="t2",
                                       tag="t2", bufs=2)
                        nc.vector.tensor_mul(t2[:], i_t[:, cs], g_t[:, cs])
                        nc.vector.tensor_add(c_t[:, cs], t1[:], t2[:])
                    nc.scalar.activation(tc_t[:, cs], c_t[:, cs], AF.Tanh)
                    nc.vector.tensor_mul(h_t[:, cs], o_t[:, cs], tc_t[:, cs])
                if t in tp_sched:
                    for tcc, js in tp_sched[t]:
                        emit_transposes(tcc, js)
                h_prev, c_prev = h_t, c_t

            # ---- tail: h half of the head ----
            emit_h_head(h_prev)

    nc.compile()
    return nc


def prep_in_maps(inputs, n_cores: int, b_loc: int):
    history = np.ascontiguousarray(np.asarray(inputs["history"], np.float32))
    cluster = np.ascontiguousarray(np.asarray(inputs["cluster_num"], np.float32))
    pref = np.ascontiguousarray(np.asarray(inputs["pref"], np.float32))
    w_ih = np.asarray(inputs["W_ih"], np.float32)
    w_hh = np.asarray(inputs["W_hh"], np.float32)
    b_ih = np.asarray(inputs["b_ih"], np.float32)
    b_hh = np.asarray(inputs["b_hh"], np.float32)
    fc_w = np.asarray(inputs["fc_w"], np.float32)
    fc_b = np.asarray(inputs["fc_b"], np.float32)

    shared = {
        "w_ih_t": np.ascontiguousarray(w_ih.T.astype(np.float16)),
        "w_hh_t": np.ascontiguousarray(w_hh.T.astype(np.float16)),
        "bias4": np.ascontiguousarray((b_ih + b_hh).reshape(G4, H).T),  # [128,4]
        "fc_w_t": np.ascontiguousarray(fc_w.T.astype(np.float16)),
        "fc_b_row": np.ascontiguousarray(fc_b.reshape(1, H).astype(np.float16)),
        "ones_row": np.ones((1, 128), np.float16),
        "ident": np.eye(128, dtype=np.float16),
    }
    in_maps = []
    for c in range(n_cores):
        r0, r1 = c * b_loc, (c + 1) * b_loc
        in_maps.append({
            "history": history[r0:r1].reshape(b_loc, T, D),
            "cluster_num": cluster[r0:r1],
            "pref": pref[r0:r1].reshape(b_loc, L, M3),
            **shared,
        })
    return in_maps


def run(inputs, n_cores: int = N_CORES, trace: bool = False):
    B = np.asarray(inputs["history"]).shape[0]
    b_loc = B // n_cores
    nc = build_program(b_loc)
    in_maps = prep_in_maps(inputs, n_cores, b_loc)
    res = bass_utils.run_bass_kernel_spmd(
        nc, in_maps, core_ids=list(range(n_cores)), trace=trace)
    outs = [res.results[c]["out"].reshape(b_loc, L, 256 * M3)
            for c in range(n_cores)]
    return np.concatenate(outs, axis=0), res


def kernel(**inputs) -> np.ndarray:
    out, _ = run(inputs, N_CORES)
    return out
